# revision 3
# baseline (speedup 1.0000x reference)
"""Fused boundary-rendering kernel for Trainium2 (8 NeuronCores, 1 launch).

For x of shape (2, 4, 64, 256, 256) f32:
    mn/mx  = per-channel global min/max
    binary = ((x - mn) / (mx - mn)) > 0.5     [== (x - mn) > 0.5*(mx - mn)]
    dilated = 3x3x3 binary dilation of binary (SAME padding)
    out    = dilated - binary

Sharding: H (=256) split into 8 chunks of 32 rows, one per NeuronCore, with
one halo row per side (global edges padded with -1e30 -> mask 0).  On-core
layout puts (B, D) = 128 on the SBUF partition axis; (C, H, W) on the free
axis.

The host pre-shards x into a per-core DRAM layout [C, 2, 128, 17*W] f32 that
is contiguous per (chunk, partition), so the SWDGE (gpsimd) DMA loads spray
all 16 SDMA engines with large packets.  Output leaves the device as
[C, 2, 128, 16*W] uint8 and is scattered back into the f32 result on host.

Single launch per core:
  1. 8 chunked SWDGE loads; per-chunk partial min/max on DVE overlapped
     with the loads.
  2. Partials transposed across partitions with a PE identity matmul,
     reduced to 8 floats [mx(4) | -mn(4)], bounced to DRAM and combined
     across the 8 cores with an AllReduce(max) collective.
  3. Result broadcast to 128 partitions with a rank-1 PE matmul; per
     channel: binary mask on DVE (tensor_scalar is_gt), H-dilated mask mH
     on DVE (2 max ops) -- both double-buffered so the PE pipeline never
     stalls between channels -- then 3x3x3 count via banded PE matmuls
     with +-1 W shifts, fused -16*binary accumulation, and a saturated
     sigmoid on ACT producing exact {0,1} uint8, stored with SWDGE DMAs.
"""

import os
import sys

import numpy as np

for _p in ("/opt/trn_rl_repo", "/root/.axon_site/_ro/trn_rl_repo"):
    if os.path.isdir(_p) and _p not in sys.path:
        sys.path.insert(0, _p)

import ml_dtypes

B, C, D, H, W = 2, 4, 64, 256, 256
NCORES = 8
HS = H // NCORES  # 32 own rows per core
HA = HS + 2  # rows incl halo
HPAD = np.float32(-1e30)  # halo pad at global H edges -> mask 0

MHW = 258  # mH row width: 256 data cols + 2 zero pad cols
MHLEN = 2 + 16 * MHW + 2  # 16-row half-buffer with 2-col lead pad + tail slack

_CACHE = {}


def _consts():
    bd = np.arange(128)
    b = bd // D
    d = bd % D
    A = (b[:, None] == b[None, :]) & (np.abs(d[:, None] - d[None, :]) <= 1)
    A = A.astype(ml_dtypes.bfloat16)
    negI = (-16.0 * np.eye(128)).astype(ml_dtypes.bfloat16)
    I128 = np.eye(128, dtype=np.float32)
    return A, negI, I128


def _build():
    import concourse.bass as bass
    import concourse.bacc as bacc
    import concourse.mybir as mybir
    import concourse.tile as tile
    from contextlib import ExitStack

    f32 = mybir.dt.float32
    bf16 = mybir.dt.bfloat16
    u8 = mybir.dt.uint8
    Alu = mybir.AluOpType

    nc = bacc.Bacc(
        "TRN2",
        target_bir_lowering=False,
        debug=False,
        num_devices=NCORES,
    )

    # [c, half, partition, 17*W] contiguous per chunk
    xs = nc.dram_tensor("xs", [C, 2, 128, 17 * W], f32, kind="ExternalInput")
    # [c, t, partition, 16*W] contiguous per store
    out = nc.dram_tensor("out", [C, 2, 128, 16 * W], u8, kind="ExternalOutput")
    A_np, negI_np, I_np = _consts()
    bandA_d = nc.inline_tensor(A_np, name="bandA")
    negI_d = nc.inline_tensor(negI_np, name="negI")
    ident_d = nc.inline_tensor(I_np, name="ident")

    xsa = xs.ap()
    outa = out.ap()

    with ExitStack() as ctx:
        tc = ctx.enter_context(tile.TileContext(nc))
        pers = ctx.enter_context(tc.tile_pool(name="pers", bufs=1))
        binp = ctx.enter_context(tc.tile_pool(name="binp", bufs=2))
        stagp = ctx.enter_context(tc.tile_pool(name="stag", bufs=2))
        psump = ctx.enter_context(tc.tile_pool(name="psum", bufs=2, space="PSUM"))
        dram = ctx.enter_context(tc.tile_pool(name="dram", bufs=1, space="DRAM"))

        x_all = pers.tile([128, C, HA, W], f32)  # 136 KiB / partition
        pmax = pers.tile([128, 12], f32)  # 3 partial slots per channel
        pmin = pers.tile([128, 12], f32)
        red8 = pers.tile([128, 8], f32)  # [mx(4) | -mn(4)] per partition
        s8 = pers.tile([128, 1], f32)  # per-partition reduced (parts 0..7)
        s1v = pers.tile([128, 8], f32)  # reduced vals on partition 0
        gv8 = pers.tile([128, 8], f32)  # broadcast [mx | -mn] on all parts
        mnv = pers.tile([128, 4], f32)  # mn per channel
        h4 = pers.tile([128, 4], f32)  # 0.5*(mx-mn) per channel
        At = pers.tile([128, 128], bf16)
        Nt = pers.tile([128, 128], bf16)
        It = pers.tile([128, 128], f32)
        ones1 = pers.tile([128, 128], f32)  # row 0 used as all-ones lhsT
        sel_bias = pers.tile([128, 1], f32)
        # manual 3-buffer rotation for the 16-row H-dilated mask halves so
        # the pad columns can be zeroed ONCE (full-tile memsets) up front
        mhs = [pers.tile([128, MHLEN], bf16, name=f"mh{i}") for i in range(3)]
        cc_in = dram.tile([8, 1], f32)
        cc_out = dram.tile([8, 1], f32)

        nc.vector.memset(sel_bias[:, :], -100.0)
        nc.vector.memset(ones1[:, :], 1.0)
        nc.sync.dma_start(out=At[:, :], in_=bandA_d.ap())
        nc.sync.dma_start(out=Nt[:, :], in_=negI_d.ap())
        nc.sync.dma_start(out=It[:, :], in_=ident_d.ap())

        # ---- phase 1: load + per-core min/max partials ----
        # 8 chunks (c, half); rows [0,17) and [17,34).  Partial reductions
        # cover own rows only (1..32); chunk boundaries overlap at row 17
        # which is harmless for min/max.
        for c in range(C):
            for half in range(2):
                if c == 0 and half == 0:
                    # split the first chunk so the DVE reductions start as
                    # early as possible (they are the phase-1 critical path)
                    nc.gpsimd.dma_start(
                        out=x_all[:, 0, 0:9, :].rearrange("p r w -> p (r w)"),
                        in_=xsa[0, 0, :, 0 : 9 * W],
                    )
                    nc.gpsimd.dma_start(
                        out=x_all[:, 0, 9:17, :].rearrange("p r w -> p (r w)"),
                        in_=xsa[0, 0, :, 9 * W :],
                    )
                    continue
                nc.gpsimd.dma_start(
                    out=x_all[:, c, 17 * half : 17 * half + 17, :].rearrange(
                        "p r w -> p (r w)"
                    ),
                    in_=xsa[c, half],
                )
        # partial min/max on DVE (tensor_reduce is 1 elem/cycle; scans and
        # gpsimd alternatives measured no faster / broken).  c=0 is split
        # finer so DVE starts as soon as the first small load lands.
        for c in range(C):
            if c == 0:
                ranges = [(1, 9), (9, 17), (17, 33)]
            else:
                ranges = [(1, 17), (17, 33)]
            for j, (lo, hi) in enumerate(ranges):
                nc.vector.tensor_reduce(
                    out=pmin[:, 3 * c + j : 3 * c + j + 1],
                    in_=x_all[:, c, lo:hi, :],
                    axis=mybir.AxisListType.XY,
                    op=Alu.min,
                )
                nc.vector.tensor_reduce(
                    out=pmax[:, 3 * c + j : 3 * c + j + 1],
                    in_=x_all[:, c, lo:hi, :],
                    axis=mybir.AxisListType.XY,
                    op=Alu.max,
                )
        # red8 = [mx(4) | -mn(4)]
        for c in range(C):
            nslots = 3 if c == 0 else 2
            nc.vector.tensor_reduce(
                out=red8[:, c : c + 1],
                in_=pmax[:, 3 * c : 3 * c + nslots],
                axis=mybir.AxisListType.X,
                op=Alu.max,
            )
            nc.vector.tensor_reduce(
                out=red8[:, 4 + c : 5 + c],
                in_=pmin[:, 3 * c : 3 * c + nslots],
                axis=mybir.AxisListType.X,
                op=Alu.min,
            )
        nc.vector.tensor_scalar_mul(red8[:, 4:8], red8[:, 4:8], -1.0)
        # cross-partition max via PE transpose + DVE free-axis reduce
        pst = psump.tile([128, 2048], f32, tag="ps")
        nc.tensor.matmul(pst[0:8, 0:128], red8[:, :], It[:, :], start=True, stop=True)
        nc.vector.tensor_reduce(
            out=s8[0:8, 0:1],
            in_=pst[0:8, 0:128],
            axis=mybir.AxisListType.X,
            op=Alu.max,
        )
        # ---- cross-core AllReduce(max) on 8 floats ----
        nc.sync.dma_start(out=cc_in[:, :], in_=s8[0:8, 0:1])
        # zero the mH buffers while the collective round-trips; their pad
        # columns stay zero for all of phase 2.  The tiny copy from s8
        # pins each memset behind the phase-1 reduces -- without it the
        # scheduler hoists them to t=0, delaying the first reduction.
        for mh_ in mhs:
            nc.vector.tensor_copy(mh_[:, 0:1], s8[:, 0:1])
            nc.vector.memset(mh_[:, :], 0.0)
        nc.gpsimd.collective_compute(
            "AllReduce",
            Alu.max,
            replica_groups=[list(range(NCORES))],
            ins=[cc_in[:, :].opt()],
            outs=[cc_out[:, :].opt()],
        )
        nc.sync.dma_start(
            out=s1v[0:1, 0:8],
            in_=cc_out[:, :].rearrange("k j -> (k j)")[None, :],
        )
        # PE warmup: dummy matmuls that fill the PE idle window while the
        # collective round-trips, so phase 2 starts at the warm 2.4 GHz
        # clock instead of the throttled 1.2 GHz (HAM re-throttles after
        # ~3.4 us idle).  They depend on s8 so they run no earlier.
        psw = psump.tile([128, 2048], f32, tag="ps")
        warm_rhs = mhs[0][:, 0 : 2 * MHW].rearrange("p (r z) -> p r z", z=MHW)[
            :, :, 0:W
        ]
        for wi in range(16):
            nc.tensor.matmul(
                psw[:, 512 * (wi % 4) : 512 * (wi % 4) + 512],
                At[:, :],
                warm_rhs,
                start=True,
                stop=True,
                skip_group_check=True,
            )
        # broadcast to all 128 partitions with a rank-1 matmul
        psb = psump.tile([128, 2048], f32, tag="ps")
        nc.tensor.matmul(psb[:, 0:8], ones1[0:1, :], s1v[0:1, 0:8], start=True, stop=True)
        # second warmup batch: keeps PE at the warm clock while DVE derives
        # the first channel's mask after the collective result lands
        for wi in range(16):
            nc.tensor.matmul(
                psw[:, 512 * (wi % 4) : 512 * (wi % 4) + 512],
                At[:, :],
                warm_rhs,
                start=True,
                stop=True,
                skip_group_check=True,
            )
        nc.vector.tensor_copy(gv8[:, :], psb[:, 0:8])
        nc.vector.tensor_scalar_mul(mnv[:, :], gv8[:, 4:8], -1.0)
        nc.vector.tensor_add(h4[:, :], gv8[:, 0:4], gv8[:, 4:8])
        nc.vector.tensor_scalar_mul(h4[:, :], h4[:, :], 0.5)

        # ---- phase 2: mask, dilate, boundary ----
        # binm and the 16-row mH half-buffers rotate so DVE prep of channel
        # c+1 overlaps the PE matmuls of channel c.
        for c in range(C):
            binm = binp.tile([128, HA, W], bf16, tag="bin")
            nc.vector.tensor_scalar(
                out=binm[:, :, :],
                in0=x_all[:, c, :, :],
                scalar1=mnv[:, c : c + 1],
                scalar2=h4[:, c : c + 1],
                op0=Alu.subtract,
                op1=Alu.is_gt,
            )
            for t in range(2):  # 16 own rows per mH half / staging buffer
                mH = mhs[(2 * c + t) % 3]
                mHd = mH[:, 2 : 2 + 16 * MHW].rearrange("p (r z) -> p r z", z=MHW)[
                    :, :, 0:W
                ]
                # H-dilation of own rows 16t..16t+15 (binm rows are offset +1)
                r0 = 16 * t
                nc.vector.tensor_tensor(
                    out=mHd,
                    in0=binm[:, r0 : r0 + 16, :],
                    in1=binm[:, r0 + 2 : r0 + 18, :],
                    op=Alu.max,
                )
                nc.vector.tensor_tensor(
                    out=mHd,
                    in0=mHd,
                    in1=binm[:, r0 + 1 : r0 + 17, :],
                    op=Alu.max,
                )
                stag = stagp.tile([128, 4096], u8, tag="st")
                ps = psump.tile([128, 2048], f32, tag="ps")
                ps2 = psump.tile([128, 2048], f32, tag="ps")
                for half, pst_ in ((0, ps), (1, ps2)):
                    for s in range(4):  # one PSUM bank = 2 rows = 512
                        Rr = 8 * half + 2 * s  # row within this half-buffer
                        R = 16 * t + Rr
                        pslice = pst_[:, 512 * s : 512 * s + 512]
                        for j, dw in enumerate((-1, 0, 1)):
                            off = 2 + Rr * MHW + dw
                            rhs = mH[:, off : off + 2 * MHW].rearrange(
                                "p (r z) -> p r z", z=MHW
                            )[:, :, 0:W]
                            nc.tensor.matmul(
                                pslice,
                                At[:, :],
                                rhs,
                                start=(j == 0),
                                stop=False,
                            )
                        nc.tensor.matmul(
                            pslice,
                            Nt[:, :],
                            binm[:, 1 + R : 3 + R, :],
                            start=False,
                            stop=True,
                        )
                    nc.scalar.activation(
                        out=stag[:, 2048 * half : 2048 * half + 2048],
                        in_=pst_[:, :],
                        func=mybir.ActivationFunctionType.Sigmoid,
                        bias=sel_bias[:, :],
                        scale=200.0,
                    )
                nc.gpsimd.dma_start(out=outa[c, t], in_=stag[:, :])

    nc.compile()
    return nc


def _get_nc():
    if "nc" not in _CACHE:
        _CACHE["nc"] = _build()
    return _CACHE["nc"]


def _make_in_maps(x: np.ndarray):
    """Pre-shard to [C, 2, 128, 17*W] f32 per core: partition p=(b,d), 17
    halo-extended rows per (c, half) chunk, contiguous per chunk."""
    in_maps = []
    # halo-extended H view per core built once:
    for k in range(NCORES):
        lo = k * HS
        xss = np.empty((C, 2, 2, D, 17, W), np.float32)  # c, half, b, d, r, w
        # rows: half 0 -> global rows lo-1 .. lo+15; half 1 -> lo+16 .. lo+32
        for half in range(2):
            glo = lo - 1 + 17 * half
            src = np.empty((B, C, D, 17, W), np.float32)
            s0, s1 = max(glo, 0), min(glo + 17, H)
            src[:, :, :, s0 - glo : s1 - glo, :] = x[:, :, :, s0:s1, :]
            if glo < 0:
                src[:, :, :, 0, :] = HPAD
            if glo + 17 > H:
                src[:, :, :, H - glo :, :] = HPAD
            xss[:, half] = src.transpose(1, 0, 2, 3, 4)
        in_maps.append({"xs": xss.reshape(C, 2, 128, 17 * W)})
    return in_maps


def kernel(x: np.ndarray) -> np.ndarray:
    from concourse.bass_utils import run_bass_kernel_spmd

    x = np.ascontiguousarray(np.asarray(x), dtype=np.float32)
    assert x.shape == (B, C, D, H, W)

    in_maps = _make_in_maps(x)
    res = run_bass_kernel_spmd(_get_nc(), in_maps, core_ids=list(range(NCORES)))
    y = np.empty((B, C, D, H, W), np.float32)
    for k in range(NCORES):
        o = res.results[k]["out"].reshape(C, 2, B, D, 16, W)
        y[:, :, :, k * HS : (k + 1) * HS, :] = (
            o.transpose(2, 0, 3, 1, 4, 5).reshape(B, C, D, HS, W).astype(np.float32)
        )
    return y


if __name__ == "__main__":
    x = np.random.randn(B, C, D, H, W).astype(np.float32)
    y = kernel(x)
    print(y.shape, y.dtype, y.sum())
